# revision 1
# baseline (speedup 1.0000x reference)
"""Trainium2 Bass kernel for nn_LowRankSTLayer_dilation.

Mathematical reduction (validated vs the jax reference, ~6e-7 absmax rel):
  1. U/V start rank-symmetric and the multiplicative NMF updates preserve
     that, so the rank-3 iteration is exactly rank-1.
  2. eps=1e-6 is negligible vs the O(1)+ denominators, so each update is a
     plain normalized projection -- power iteration on the per-position
     Gram matrix G = X X^T.  All normalization scalars cancel:
         out = relu( tail_w @ ( p3 * <h,p2>/<p3,p2> ) )
     p0 = box27(h), p_{n+1} = G p_n, G = box27(h h^T) (separable 3x3x3
     box filter of the 136 channel-pair products), h = relu(head_w @ x).

Sharding: 8 cores = batch(2) x frame-pairs(4); each core receives a
replicate-padded slice [17, 4, 98, 98] (channel 16 is constant 1.0, used
to synthesize constant rows via PE matmuls).
"""

import numpy as np
from contextlib import ExitStack

import concourse.bass as bass
import concourse.bacc as bacc
import concourse.tile as tile
from concourse import mybir
from concourse.bass_utils import run_bass_kernel_spmd

F32 = mybir.dt.float32
F32R = mybir.dt.float32r

B, C, D, H, W = 2, 16, 8, 96, 96
NCORES = 8
HP, WP = H + 2, W + 2            # spatially padded
DF = 4                            # frames per core incl. temporal halo
R = 8                             # output rows per chunk
NCHUNK = H // R                   # 12
RIN = R + 2                       # input rows per chunk
PIN_F = RIN * WP                  # input positions per frame per chunk (980)
PIECE = PIN_F // 2                # matmul piece (490 <= 512)
POS = 2 * R * W                   # output positions per chunk (1536)
T2P = 40                          # tile-2 partitions (32..39 = diag-hi)
CPIECE = 512
NPAIR = 120

_pairs = [(a, b) for a in range(C) for b in range(a + 1, C)]
_A = np.array([p[0] for p in _pairs])
_B = np.array([p[1] for p in _pairs])


def _build_consts(head_w, tail_w):
    hwT = head_w.T.astype(np.float32)          # [c_in, c_out]
    # head conv + ones passthrough: h16ext = relu([head_w @ x ; x_ones])
    w_head = np.zeros((C + 1, C + 1), np.float32)
    w_head[:C, :C] = hwT
    w_head[C, C] = 1.0
    # 0/1 selectors over relu'd h16ext channels (row 16 = const 1.0)
    w_a = np.zeros((C + 1, 128), np.float32)
    w_b = np.zeros((C + 1, 128), np.float32)
    w_a[_A, np.arange(NPAIR)] = 1.0
    w_b[_B, np.arange(NPAIR)] = 1.0
    w_a[np.arange(8), NPAIR + np.arange(8)] = 1.0   # diag-lo: h[0..7]^2
    w_b[np.arange(8), NPAIR + np.arange(8)] = 1.0
    # tile-2: partitions 0..15 = h (x ones), 32..39 = h[8..15]^2
    w_f = np.zeros((C + 1, T2P), np.float32)
    w_g = np.zeros((C + 1, T2P), np.float32)
    w_f[np.arange(C), np.arange(16)] = 1.0
    w_g[C, :16] = 1.0
    w_f[8 + np.arange(8), 32 + np.arange(8)] = 1.0
    w_g[8 + np.arange(8), 32 + np.arange(8)] = 1.0
    sel_b = np.zeros((C, 128), np.float32)
    sel_b[_B, np.arange(NPAIR)] = 1.0
    sel_b[np.arange(8), NPAIR + np.arange(8)] = 1.0
    sel_a = np.zeros((C, NPAIR), np.float32)
    sel_a[_A, np.arange(NPAIR)] = 1.0
    sel_h = np.zeros((C, T2P), np.float32)
    sel_h[8 + np.arange(8), 32 + np.arange(8)] = 1.0
    s_a = np.zeros((128, C), np.float32)
    s_a[np.arange(NPAIR), _A] = 1.0
    s_a[NPAIR + np.arange(8), np.arange(8)] = 1.0
    s_b = np.zeros((NPAIR, C), np.float32)
    s_b[np.arange(NPAIR), _B] = 1.0
    s_h = np.zeros((T2P, C), np.float32)
    s_h[32 + np.arange(8), 8 + np.arange(8)] = 1.0
    ones_a = np.ones((C, 1), np.float32)
    ones_g = np.ones((1, C), np.float32)
    tail_t = tail_w.T.astype(np.float32).copy()
    return dict(w_head=w_head, w_a=w_a, w_b=w_b, w_f=w_f, w_g=w_g,
                sel_b=sel_b,
                sel_a=sel_a, sel_h=sel_h, s_a=s_a, s_b=s_b, s_h=s_h,
                ones_a=ones_a, ones_g=ones_g, tail_t=tail_t)


_CONST_SHAPES = dict(w_head=(C + 1, C + 1), w_a=(C + 1, 128),
                     w_b=(C + 1, 128), w_f=(C + 1, T2P),
                     w_g=(C + 1, T2P), sel_b=(C, 128), sel_a=(C, NPAIR),
                     sel_h=(C, T2P), s_a=(128, C), s_b=(NPAIR, C),
                     s_h=(T2P, C), ones_a=(C, 1), ones_g=(1, C),
                     tail_t=(C, C))


def _build_program():
    nc = bacc.Bacc("TRN2", target_bir_lowering=False, debug=False)
    xin = nc.declare_dram_parameter("xin", [C + 1, DF, HP, WP], F32R,
                                    isOutput=False)
    cst = {k: nc.declare_dram_parameter(k, list(v), F32R, isOutput=False)
           for k, v in _CONST_SHAPES.items()}
    out = nc.declare_dram_parameter("out", [C, 2, H, W], F32, isOutput=True)

    def mmr(out_, lhsT, rhs, **kw):
        nc.tensor.matmul(out_, lhsT.bitcast(F32R), rhs.bitcast(F32R), **kw)

    with tile.TileContext(nc) as tc, ExitStack() as ctx:
        singles = ctx.enter_context(tc.tile_pool(name="singles", bufs=1))
        sb = {}
        for k, v in _CONST_SHAPES.items():
            sb[k] = singles.tile(list(v), F32R, tag=k, name=k)
            nc.sync.dma_start(out=sb[k], in_=cst[k][:, :])

        xpool = ctx.enter_context(tc.tile_pool(name="x", bufs=2))
        ps = ctx.enter_context(tc.tile_pool(name="ps", bufs=2, space="PSUM"))
        mpool = ctx.enter_context(tc.tile_pool(name="m", bufs=1))
        boxp = ctx.enter_context(tc.tile_pool(name="box", bufs=1))
        gap = ctx.enter_context(tc.tile_pool(name="gap", bufs=1))
        pp = ctx.enter_context(tc.tile_pool(name="pp", bufs=1))
        outp = ctx.enter_context(tc.tile_pool(name="outp", bufs=1))
        gmp = ctx.enter_context(tc.tile_pool(name="gmp", bufs=2))

        for ci in range(NCHUNK):
            r0 = ci * R            # first (halo) padded row of the chunk
            # ---- pair-product tiles (pre-box) ----
            m1 = mpool.tile([128, DF, PIN_F], F32, tag="m1")
            m2 = mpool.tile([T2P, DF, PIN_F], F32, tag="m2")
            for f in range(DF):
                xs = xpool.tile([C + 1, RIN, WP], F32R)
                nc.sync.dma_start(out=xs, in_=xin[:, f, r0:r0 + RIN, :])
                xf = xs.rearrange("c r w -> c (r w)")
                for pc in range(2):
                    sl = slice(pc * PIECE, (pc + 1) * PIECE)
                    ph = ps.tile([C + 1, PIECE], F32, tag="q0")
                    mmr(ph, sb["w_head"], xf[:, sl],
                                     start=True, stop=True)
                    hx = xpool.tile([C + 1, PIECE], F32R, tag="hx")
                    nc.scalar.activation(hx, ph,
                                         mybir.ActivationFunctionType.Relu)
                    pa = ps.tile([128, PIECE], F32, tag="q1")
                    mmr(pa, sb["w_a"], hx,
                                     start=True, stop=True)
                    ha = xpool.tile([128, PIECE], F32, tag="ha")
                    nc.scalar.copy(ha, pa)
                    pb = ps.tile([128, PIECE], F32, tag="q2")
                    mmr(pb, sb["w_b"], hx,
                                     start=True, stop=True)
                    nc.vector.tensor_mul(m1[:, f, sl], ha, pb)
                    pf = ps.tile([T2P, PIECE], F32, tag="q3")
                    mmr(pf, sb["w_f"], hx,
                                     start=True, stop=True)
                    hf = xpool.tile([T2P, PIECE], F32, tag="hf")
                    nc.scalar.copy(hf, pf)
                    pg = ps.tile([T2P, PIECE], F32, tag="q0")
                    mmr(pg, sb["w_g"], hx,
                                     start=True, stop=True)
                    nc.vector.tensor_mul(m2[:, f, sl], hf, pg)

            # ---- separable box filter: d -> i -> j ----
            def box(src, parts, eng, odt=F32):
                v = src.rearrange("p f (r w) -> p f r w", w=WP)
                t0 = boxp.tile([parts, 2, RIN, WP], F32, tag=f"tmp{parts}")
                bd = boxp.tile([parts, 2, RIN, WP], F32, tag=f"bd{parts}")
                eng.tensor_add(t0, v[:, 0:2], v[:, 1:3])
                eng.tensor_add(bd, t0, v[:, 2:4])
                t1 = boxp.tile([parts, 2, R, WP], F32, tag=f"tmp{parts}")
                bi = boxp.tile([parts, 2, R, WP], F32, tag=f"bi{parts}")
                eng.tensor_add(t1, bd[:, :, 0:R], bd[:, :, 1:R + 1])
                eng.tensor_add(bi, t1, bd[:, :, 2:R + 2])
                t2 = boxp.tile([parts, 2, R, W], F32, tag=f"tmp{parts}")
                bj = boxp.tile([parts, 2, R, W], odt, tag=f"bj{parts}")
                eng.tensor_add(t2, bi[:, :, :, 0:W], bi[:, :, :, 1:W + 1])
                eng.tensor_add(bj, t2, bi[:, :, :, 2:W + 2])
                return bj

            g1 = box(m1, 128, nc.vector)
            g2 = box(m2, T2P, nc.gpsimd, odt=F32R)      # [0:16]=p0, [32:40]=diag-hi
            g1v = g1.rearrange("p f r w -> p (f r w)")
            g2v = g2.rearrange("p f r w -> p (f r w)")

            # ---- power iteration: p_{n+1} = G p_n ----
            p_bufs = []
            p_cur = g2v[0:16, :]
            for app in range(3):
                pia = gap.tile([128, POS], F32R, tag="pia")
                pib = gap.tile([NPAIR, POS], F32R, tag="pib")
                pih = gap.tile([T2P, POS], F32R, tag="pih")
                pnx = pp.tile([16, POS], F32R, tag=f"p{app}")
                for pc in range(POS // CPIECE):
                    sl = slice(pc * CPIECE, (pc + 1) * CPIECE)
                    prb = ps.tile([128, CPIECE], F32, tag="q0")
                    pra = ps.tile([NPAIR, CPIECE], F32, tag="q1")
                    prh = ps.tile([T2P, CPIECE], F32, tag="q2")
                    mmr(prb, sb["sel_b"], p_cur[:, sl],
                                     start=True, stop=True)
                    mmr(pra, sb["sel_a"], p_cur[:, sl],
                                     start=True, stop=True)
                    mmr(prh, sb["sel_h"], p_cur[:, sl],
                                     start=True, stop=True)
                    nc.vector.tensor_mul(pia[:, sl], g1v[:, sl], prb)
                    nc.vector.tensor_mul(pib[:, sl], g1v[0:NPAIR, sl], pra)
                    nc.vector.tensor_mul(pih[32:40, sl],
                                         g2v[32:40, sl].bitcast(F32),
                                         prh[32:40, :])
                    acc = ps.tile([16, CPIECE], F32, tag="q3")
                    mmr(acc, sb["s_a"], pia[:, sl],
                                     start=True, stop=False)
                    mmr(acc, sb["s_b"], pib[:, sl],
                                     start=False, stop=False)
                    mmr(acc, sb["s_h"][32:40, :],
                                     pih[32:40, sl], start=False, stop=True)
                    nc.scalar.copy(pnx[:, sl], acc)
                p_bufs.append(pnx)
                p_cur = pnx
            p2, p3 = p_bufs[1], p_bufs[2]

            # ---- gamma = <h,p2>/<p3,p2>; out = relu(tail (gamma*p3)) ----
            hcore = m2[0:16, 1:3, :].rearrange(
                "c f (r w) -> c f r w", w=WP)[:, :, 1:R + 1, 1:W + 1]
            thn = gap.tile([16, 2, R, W], F32R, tag="thn")
            tdn = gap.tile([16, POS], F32R, tag="tdn")
            nc.vector.tensor_mul(
                thn, hcore,
                p2.rearrange("c (f r w) -> c f r w", f=2, r=R).bitcast(F32))
            nc.vector.tensor_mul(tdn, p3.bitcast(F32), p2.bitcast(F32))
            thnv = thn.rearrange("c f r w -> c (f r w)")
            osb = outp.tile([16, POS], F32, tag="osb")
            for pc in range(POS // CPIECE):
                sl = slice(pc * CPIECE, (pc + 1) * CPIECE)
                pnum = ps.tile([1, CPIECE], F32, tag="q0")
                pden = ps.tile([1, CPIECE], F32, tag="q1")
                mmr(pnum, sb["ones_a"], thnv[:, sl],
                                 start=True, stop=True)
                mmr(pden, sb["ones_a"], tdn[:, sl],
                                 start=True, stop=True)
                gam = gmp.tile([1, CPIECE], F32R, tag="gam")
                rcp = gmp.tile([1, CPIECE], F32, tag="rcp")
                lnd = gmp.tile([1, CPIECE], F32, tag="lnd")
                nc.scalar.activation(lnd, pden,
                                     mybir.ActivationFunctionType.Ln)
                nc.scalar.activation(rcp, lnd,
                                     mybir.ActivationFunctionType.Exp,
                                     scale=-1.0)
                nc.vector.tensor_mul(gam, pnum, rcp)
                grep = ps.tile([16, CPIECE], F32, tag="q2")
                mmr(grep, sb["ones_g"], gam,
                                 start=True, stop=True)
                upre = gap.tile([16, CPIECE], F32R, tag="upre")
                nc.vector.tensor_mul(upre, p3[:, sl].bitcast(F32), grep)
                pout = ps.tile([16, CPIECE], F32, tag="q3")
                mmr(pout, sb["tail_t"], upre,
                                 start=True, stop=True)
                nc.scalar.activation(osb[:, sl], pout,
                                     mybir.ActivationFunctionType.Relu)
            nc.sync.dma_start(
                out=out[:, :, ci * R:(ci + 1) * R, :],
                in_=osb.rearrange("c (f r w) -> c f r w", f=2, r=R))
    nc.compile()
    return nc


_NC_CACHE = None
TRACE = False
LAST_EXEC_NS = None


def kernel(x, head_w, tail_w):
    global _NC_CACHE, LAST_EXEC_NS
    x = np.asarray(x, dtype=np.float32)
    head_w = np.asarray(head_w, dtype=np.float32)
    tail_w = np.asarray(tail_w, dtype=np.float32)

    consts = _build_consts(head_w, tail_w)
    xp = np.pad(x, ((0, 0), (0, 0), (1, 1), (1, 1), (1, 1)), mode="edge")
    in_maps = []
    for core in range(NCORES):
        b, q = divmod(core, 4)
        xs = np.empty((C + 1, DF, HP, WP), np.float32)
        xs[:C] = xp[b, :, 2 * q:2 * q + DF]
        xs[C] = 1.0
        m = {"xin": xs}
        m.update(consts)
        in_maps.append(m)

    if _NC_CACHE is None:
        _NC_CACHE = _build_program()
    res = run_bass_kernel_spmd(_NC_CACHE, in_maps, list(range(NCORES)),
                               trace=TRACE)
    LAST_EXEC_NS = res.exec_time_ns

    outf = np.empty((B, C, D, H, W), np.float32)
    for core in range(NCORES):
        b, q = divmod(core, 4)
        outf[b, :, 2 * q:2 * q + 2] = res.results[core]["out"]
    return outf



# revision 4
# speedup vs baseline: 1.2846x; 1.2846x over previous
"""Trainium2 Bass kernel for nn_LowRankSTLayer_dilation.

Mathematical reduction (validated vs the jax reference, ~6e-7 absmax rel):
  1. U/V start rank-symmetric and the multiplicative NMF updates preserve
     that, so the rank-3 iteration is exactly rank-1.
  2. eps=1e-6 is negligible vs the O(1)+ denominators, so each update is a
     plain normalized projection -- power iteration on the per-position
     Gram matrix G = X X^T.  All normalization scalars cancel:
         out = relu( tail_w @ ( p3 * <h,p2>/<p3,p2> ) )
     p0 = box27(h), p_{n+1} = G p_n, G = box27(h h^T) (separable 3x3x3
     box filter of the 136 channel-pair products), h = relu(head_w @ x).

Sharding: 8 cores = batch(2) x frame-pairs(4); each core receives a
replicate-padded slice [17, 4, 98, 98] (channel 16 is constant 1.0, used
to synthesize constant rows via PE matmuls).

bf16 pipeline: inputs, stationaries and all elementwise tiles are bf16
(PSUM accumulation stays fp32).  m2 is a 128-row tile with h in rows
0..15 and the diag-hi squares in rows 120..127 so the a-role/diag-hi
broadcast and reduction matmuls merge into single 128-wide matmuls.
"""

import numpy as np
import ml_dtypes
from contextlib import ExitStack

import concourse.bass as bass
import concourse.bacc as bacc
import concourse.tile as tile
from concourse import mybir
from concourse.bass_utils import run_bass_kernel_spmd

F32 = mybir.dt.float32
BF16 = mybir.dt.bfloat16

B, C, D, H, W = 2, 16, 8, 96, 96
NCORES = 8
HP, WP = H + 2, W + 2            # spatially padded
DF = 4                            # frames per core incl. temporal halo
R = 8                             # output rows per chunk
NCHUNK = H // R                   # 12
RIN = R + 2                       # input rows per chunk
PIN_F = RIN * WP                  # input positions per frame per chunk (980)
PIECE = PIN_F // 2                # matmul piece (490 <= 512)
POS = 2 * R * W                   # output positions per chunk (1536)
CPIECE = 512
NPAIR = 120

_pairs = [(a, b) for a in range(C) for b in range(a + 1, C)]
_A = np.array([p[0] for p in _pairs])
_B = np.array([p[1] for p in _pairs])


def _build_consts(head_w, tail_w):
    hwT = head_w.T.astype(np.float32)          # [c_in, c_out]
    # head conv + ones passthrough: h16ext = relu([head_w @ x ; x_ones])
    w_head = np.zeros((C + 1, C + 1), np.float32)
    w_head[:C, :C] = hwT
    w_head[C, C] = 1.0
    # 0/1 selectors over relu'd h16ext channels (row 16 = const 1.0)
    # m1 rows: 0..119 pair products, 120..127 diag-lo squares (c=0..7)
    w_a = np.zeros((C + 1, 128), np.float32)
    w_b = np.zeros((C + 1, 128), np.float32)
    w_a[_A, np.arange(NPAIR)] = 1.0
    w_b[_B, np.arange(NPAIR)] = 1.0
    w_a[np.arange(8), NPAIR + np.arange(8)] = 1.0
    w_b[np.arange(8), NPAIR + np.arange(8)] = 1.0
    # m2 rows: 0..15 h (x ones), 96..103 diag-hi squares (c=8..15),
    # 104..127 duplicate pair products #96..119 (so the a-role mul over
    # rows 96..128 reads a single tile at a legal partition base).
    w_f = np.zeros((C + 1, 128), np.float32)
    w_g = np.zeros((C + 1, 128), np.float32)
    w_f[np.arange(C), np.arange(16)] = 1.0
    w_g[C, :16] = 1.0
    w_f[8 + np.arange(8), 96 + np.arange(8)] = 1.0
    w_g[8 + np.arange(8), 96 + np.arange(8)] = 1.0
    w_f[_A[96:], 104 + np.arange(24)] = 1.0
    w_g[_B[96:], 104 + np.arange(24)] = 1.0
    # power-iter broadcasts: pia rows <- p[b] / p[c<8]; pibh rows <- p[a] / p[c>=8]
    sel_b = np.zeros((C, 128), np.float32)
    sel_b[_B, np.arange(NPAIR)] = 1.0
    sel_b[np.arange(8), NPAIR + np.arange(8)] = 1.0
    sel_ah = np.zeros((C, 128), np.float32)
    sel_ah[_A[:96], np.arange(96)] = 1.0
    sel_ah[8 + np.arange(8), 96 + np.arange(8)] = 1.0
    sel_ah[_A[96:], 104 + np.arange(24)] = 1.0
    # reductions: pia -> channel a / c<8 ; pibh -> channel b / c>=8
    s_a = np.zeros((128, C), np.float32)
    s_a[np.arange(NPAIR), _A] = 1.0
    s_a[NPAIR + np.arange(8), np.arange(8)] = 1.0
    s_bh = np.zeros((128, C), np.float32)
    s_bh[np.arange(96), _B[:96]] = 1.0
    s_bh[96 + np.arange(8), 8 + np.arange(8)] = 1.0
    s_bh[104 + np.arange(24), _B[96:]] = 1.0
    ones_a = np.ones((C, 1), np.float32)
    ones_g = np.ones((1, C), np.float32)
    tail_t = tail_w.T.astype(np.float32).copy()
    return dict(w_head=w_head, w_a=w_a, w_b=w_b, w_f=w_f, w_g=w_g,
                sel_b=sel_b, sel_ah=sel_ah, s_a=s_a, s_bh=s_bh,
                ones_a=ones_a, ones_g=ones_g, tail_t=tail_t)


_CONST_SHAPES = dict(w_head=(C + 1, C + 1), w_a=(C + 1, 128),
                     w_b=(C + 1, 128), w_f=(C + 1, 128),
                     w_g=(C + 1, 128), sel_b=(C, 128), sel_ah=(C, 128),
                     s_a=(128, C), s_bh=(128, C),
                     ones_a=(C, 1), ones_g=(1, C),
                     tail_t=(C, C))


def _build_program():
    nc = bacc.Bacc("TRN2", target_bir_lowering=False, debug=False)
    xin = nc.declare_dram_parameter("xin", [C + 1, DF, HP, WP], BF16,
                                    isOutput=False)
    cst = {k: nc.declare_dram_parameter(k, list(v), BF16, isOutput=False)
           for k, v in _CONST_SHAPES.items()}
    out = nc.declare_dram_parameter("out", [C, 2, H, W], F32, isOutput=True)

    with tile.TileContext(nc) as tc, ExitStack() as ctx:
        singles = ctx.enter_context(tc.tile_pool(name="singles", bufs=1))
        sb = {}
        for k, v in _CONST_SHAPES.items():
            sb[k] = singles.tile(list(v), BF16, tag=k, name=k)
            nc.sync.dma_start(out=sb[k], in_=cst[k][:, :])

        xpool = ctx.enter_context(tc.tile_pool(name="x", bufs=2))
        ps = ctx.enter_context(tc.tile_pool(name="ps", bufs=2, space="PSUM"))
        mpool = ctx.enter_context(tc.tile_pool(name="m", bufs=2))
        boxp = ctx.enter_context(tc.tile_pool(name="box", bufs=2))
        gap = ctx.enter_context(tc.tile_pool(name="gap", bufs=2))
        pp = ctx.enter_context(tc.tile_pool(name="pp", bufs=2))
        outp = ctx.enter_context(tc.tile_pool(name="outp", bufs=2))
        gmp = ctx.enter_context(tc.tile_pool(name="gmp", bufs=2))

        for ci in range(NCHUNK):
            r0 = ci * R            # first (halo) padded row of the chunk
            # ---- pair-product tiles (pre-box) ----
            m1 = mpool.tile([128, DF, PIN_F], BF16, tag="m1")
            m2 = mpool.tile([128, DF, PIN_F], BF16, tag="m2")
            for f in range(DF):
                xs = xpool.tile([C + 1, RIN, WP], BF16)
                nc.sync.dma_start(out=xs, in_=xin[:, f, r0:r0 + RIN, :])
                xf = xs.rearrange("c r w -> c (r w)")
                for pc in range(2):
                    sl = slice(pc * PIECE, (pc + 1) * PIECE)
                    ph = ps.tile([C + 1, PIECE], F32, tag="q0")
                    nc.tensor.matmul(ph, sb["w_head"], xf[:, sl],
                                     start=True, stop=True)
                    hx = xpool.tile([C + 1, PIECE], BF16, tag="hx")
                    nc.scalar.activation(hx, ph,
                                         mybir.ActivationFunctionType.Relu)
                    pa = ps.tile([128, PIECE], F32, tag="q1")
                    nc.tensor.matmul(pa, sb["w_a"], hx,
                                     start=True, stop=True)
                    ha = xpool.tile([128, PIECE], BF16, tag="ha")
                    nc.scalar.copy(ha, pa)
                    pb = ps.tile([128, PIECE], F32, tag="q2")
                    nc.tensor.matmul(pb, sb["w_b"], hx,
                                     start=True, stop=True)
                    nc.vector.tensor_mul(m1[:, f, sl], ha, pb)
                    pf = ps.tile([128, PIECE], F32, tag="q3")
                    nc.tensor.matmul(pf, sb["w_f"], hx,
                                     start=True, stop=True)
                    hf = xpool.tile([128, PIECE], BF16, tag="hf")
                    nc.scalar.copy(hf, pf)
                    pg = ps.tile([128, PIECE], F32, tag="q0")
                    nc.tensor.matmul(pg, sb["w_g"], hx,
                                     start=True, stop=True)
                    nc.vector.tensor_mul(m2[:, f, sl], hf, pg)

            # ---- separable box filter: d (gpsimd for m2) -> i -> j ----
            def box(src, tagp, eng_d, eng_i, eng_j):
                v = src.rearrange("p f (r w) -> p f r w", w=WP)
                t0 = boxp.tile([128, 2, RIN, WP], BF16, tag=f"tmp{tagp}")
                bd = boxp.tile([128, 2, RIN, WP], BF16, tag=f"bd{tagp}")
                eng_d.tensor_add(t0, v[:, 0:2], v[:, 1:3])
                eng_d.tensor_add(bd, t0, v[:, 2:4])
                t1 = boxp.tile([128, 2, R, WP], BF16, tag=f"tmp{tagp}")
                bi = boxp.tile([128, 2, R, WP], BF16, tag=f"bi{tagp}")
                eng_i.tensor_add(t1, bd[:, :, 0:R], bd[:, :, 1:R + 1])
                eng_i.tensor_add(bi, t1, bd[:, :, 2:R + 2])
                t2 = boxp.tile([128, 2, R, W], BF16, tag=f"tmp{tagp}")
                bj = boxp.tile([128, 2, R, W], BF16, tag=f"bj{tagp}")
                eng_j.tensor_add(t2, bi[:, :, :, 0:W], bi[:, :, :, 2:W + 2])
                eng_j.tensor_add(bj, t2, bi[:, :, :, 1:W + 1])
                return bj

            g1 = box(m1, "1", nc.vector, nc.vector, nc.vector)
            g2 = box(m2, "2", nc.gpsimd, nc.gpsimd, nc.vector)
            g1v = g1.rearrange("p f r w -> p (f r w)")
            g2v = g2.rearrange("p f r w -> p (f r w)")

            # ---- power iteration: p_{n+1} = G p_n ----
            p_bufs = []
            p_cur = g2v[0:16, :]
            for app in range(3):
                pia = gap.tile([128, POS], BF16, tag="pia")
                pibh = gap.tile([128, POS], BF16, tag="pibh")
                pnx = pp.tile([16, POS], BF16, tag=f"p{app}")
                for pc in range(POS // CPIECE):
                    sl = slice(pc * CPIECE, (pc + 1) * CPIECE)
                    prb = ps.tile([128, CPIECE], F32, tag="q0")
                    prab = ps.tile([128, CPIECE], F32, tag="q1")
                    nc.tensor.matmul(prb, sb["sel_b"], p_cur[:, sl],
                                     start=True, stop=True)
                    nc.tensor.matmul(prab, sb["sel_ah"], p_cur[:, sl],
                                     start=True, stop=True)
                    nc.vector.tensor_mul(pia[:, sl], g1v[:, sl], prb)
                    nc.vector.tensor_mul(pibh[0:96, sl],
                                         g1v[0:96, sl], prab[0:96, :])
                    nc.vector.tensor_mul(pibh[96:128, sl],
                                         g2v[96:128, sl], prab[96:128, :])
                    acc = ps.tile([16, CPIECE], F32, tag="q2")
                    nc.tensor.matmul(acc, sb["s_a"], pia[:, sl],
                                     start=True, stop=False)
                    nc.tensor.matmul(acc, sb["s_bh"], pibh[:, sl],
                                     start=False, stop=True)
                    nc.scalar.copy(pnx[:, sl], acc)
                p_bufs.append(pnx)
                p_cur = pnx
            p2, p3 = p_bufs[1], p_bufs[2]

            # ---- gamma = <h,p2>/<p3,p2>; out = relu(tail (gamma*p3)) ----
            hcore = m2[0:16, 1:3, :].rearrange(
                "c f (r w) -> c f r w", w=WP)[:, :, 1:R + 1, 1:W + 1]
            thn = gap.tile([16, 2, R, W], BF16, tag="thn")
            tdn = gap.tile([16, POS], BF16, tag="tdn")
            nc.vector.tensor_mul(
                thn, hcore,
                p2.rearrange("c (f r w) -> c f r w", f=2, r=R))
            nc.vector.tensor_mul(tdn, p3, p2)
            thnv = thn.rearrange("c f r w -> c (f r w)")
            osb = outp.tile([16, POS], F32, tag="osb")
            for pc in range(POS // CPIECE):
                sl = slice(pc * CPIECE, (pc + 1) * CPIECE)
                pnum = ps.tile([1, CPIECE], F32, tag="q0")
                pden = ps.tile([1, CPIECE], F32, tag="q1")
                nc.tensor.matmul(pnum, sb["ones_a"], thnv[:, sl],
                                 start=True, stop=True)
                nc.tensor.matmul(pden, sb["ones_a"], tdn[:, sl],
                                 start=True, stop=True)
                gam = gmp.tile([1, CPIECE], BF16, tag="gam")
                rcp = gmp.tile([1, CPIECE], F32, tag="rcp")
                nc.vector.reciprocal_approx_fast(out=rcp, in_=pden)
                nc.vector.tensor_mul(gam, pnum, rcp)
                grep = ps.tile([16, CPIECE], F32, tag="q2")
                nc.tensor.matmul(grep, sb["ones_g"], gam,
                                 start=True, stop=True)
                upre = gap.tile([16, CPIECE], BF16, tag="upre")
                nc.vector.tensor_mul(upre, p3[:, sl], grep)
                pout = ps.tile([16, CPIECE], F32, tag="q3")
                nc.tensor.matmul(pout, sb["tail_t"], upre,
                                 start=True, stop=True)
                nc.scalar.activation(osb[:, sl], pout,
                                     mybir.ActivationFunctionType.Relu)
            nc.sync.dma_start(
                out=out[:, :, ci * R:(ci + 1) * R, :],
                in_=osb.rearrange("c (f r w) -> c f r w", f=2, r=R))
    nc.compile()
    return nc


_NC_CACHE = None
TRACE = False
LAST_EXEC_NS = None


def kernel(x, head_w, tail_w):
    global _NC_CACHE, LAST_EXEC_NS
    x = np.asarray(x, dtype=np.float32)
    head_w = np.asarray(head_w, dtype=np.float32)
    tail_w = np.asarray(tail_w, dtype=np.float32)

    consts = {k: v.astype(ml_dtypes.bfloat16)
              for k, v in _build_consts(head_w, tail_w).items()}
    xp = np.pad(x, ((0, 0), (0, 0), (1, 1), (1, 1), (1, 1)), mode="edge")
    in_maps = []
    for core in range(NCORES):
        b, q = divmod(core, 4)
        xs = np.empty((C + 1, DF, HP, WP), ml_dtypes.bfloat16)
        xs[:C] = xp[b, :, 2 * q:2 * q + DF]
        xs[C] = 1.0
        m = {"xin": xs}
        m.update(consts)
        in_maps.append(m)

    if _NC_CACHE is None:
        _NC_CACHE = _build_program()
    res = run_bass_kernel_spmd(_NC_CACHE, in_maps, list(range(NCORES)),
                               trace=TRACE)
    LAST_EXEC_NS = res.exec_time_ns

    outf = np.empty((B, C, D, H, W), np.float32)
    for core in range(NCORES):
        b, q = divmod(core, 4)
        outf[b, :, 2 * q:2 * q + 2] = res.results[core]["out"]
    return outf
